# revision 1
# baseline (speedup 1.0000x reference)
"""Trainium2 Bass kernel for nn_ChebyshevSameInfluenceProcess.

Strategy (pure data parallel, 4 sequences per core on 8 cores):
  - Pairwise MLP term: for each sequence, pair differences x[i,k] = t_i - t_k
    are built as 2D tiles (i on partitions, k on free), flattened into
    dual-stream strips [2, N] (stream A rows i<64-block, stream B the other),
    and pushed through the 1->64->64->64->1 tanh/softplus MLP with
    block-diagonal duplicated weights so both streams use the full 128-wide
    PE array and 128 ACT lanes.  fp32r matmuls (full fp32, 1 cycle/row).
  - z4 pre-activations return to [i, k] 2D layout via DMA; softplus + static
    triangular mask + fused row-reduce gives lambda_i; runtime length masks,
    Ln, and a ones-matmul give sum(log lambda).
  - Integral term: Chebyshev coefficients of f (reference's chebft/chint,
    truncated to J=128 terms; the tail is fp32 noise) are computed on-device
    from the MLP at the 1000 Chebyshev nodes + a host-provided cos DCT matrix.
    T_j(y) for all 1024 events is built by a block-doubling ladder
    (T_{m+r} = 2 T_m T_r - T_{m-r}) using selector/anti-diagonal matmuls, then
    one K=128 matmul against cint gives all integrals.
  - Each core writes a partial (sum_log - sum_int); host sums and scales.
"""

import numpy as np

import concourse.bacc as bacc
import concourse.bass as bass
import concourse.tile as tile
import concourse.mybir as mybir
from concourse.tile import add_dep_helper

f32 = mybir.dt.float32
f32r = mybir.dt.float32r
bf16 = mybir.dt.bfloat16
i32 = mybir.dt.int32
AF = mybir.ActivationFunctionType
ALU = mybir.AluOpType

T_END = 50.0
N_NODES = 1000
J = 128          # chebyshev terms kept (reference uses 1000; tail is noise)
NSEQ = 4         # sequences per core
L = 256          # max sequence length
CON = 0.25 * T_END


def build_program():
    nc = bacc.Bacc("TRN2", target_bir_lowering=False, debug=False)

    tseq = nc.dram_tensor("tseq", (NSEQ, L), f32, kind="ExternalInput")
    lens_col = nc.dram_tensor("lens_col", (128, NSEQ), f32, kind="ExternalInput")
    W1 = nc.dram_tensor("W1", (1, 64), f32, kind="ExternalInput")
    b1 = nc.dram_tensor("b1", (64,), f32, kind="ExternalInput")
    W2 = nc.dram_tensor("W2", (64, 64), f32, kind="ExternalInput")
    b2 = nc.dram_tensor("b2", (64,), f32, kind="ExternalInput")
    W3 = nc.dram_tensor("W3", (64, 64), f32, kind="ExternalInput")
    b3 = nc.dram_tensor("b3", (64,), f32, kind="ExternalInput")
    W4 = nc.dram_tensor("W4", (64, 1), f32, kind="ExternalInput")
    b4 = nc.dram_tensor("b4", (1,), f32, kind="ExternalInput")
    bg = nc.dram_tensor("background", (1,), f32, kind="ExternalInput")
    node_xs = nc.dram_tensor("node_xs", (2, 512), f32, kind="ExternalInput")
    cosM = nc.dram_tensor("cosM", (128, 8, J + 3), f32, kind="ExternalInput")
    recj = nc.dram_tensor("recj", (1, J - 1), f32, kind="ExternalInput")
    facj = nc.dram_tensor("facj", (1, J - 1), f32, kind="ExternalInput")
    out_part = nc.dram_tensor("out_part", (1, 1), f32, kind="ExternalOutput")

    with tile.TileContext(nc) as tc:
        with (
            tc.tile_pool(name="cst", bufs=1) as cst,
            tc.tile_pool(name="x2dp", bufs=2) as x2dp,
            tc.tile_pool(name="hp", bufs=2) as hp,
            tc.tile_pool(name="fbp", bufs=2) as fbp,
            tc.tile_pool(name="ladp", bufs=2) as ladp,
            tc.tile_pool(name="pp", bufs=3, space="PSUM") as pp,
            tc.tile_pool(name="pz", bufs=1, space="PSUM") as pzp,
        ):
            # ---------------- constants / weights assembly ----------------
            W1d = cst.tile([2, 128], f32)
            nc.vector.memset(W1d[:], 0.0)
            nc.sync.dma_start(W1d[0:1, 0:64], W1[:])
            nc.sync.dma_start(W1d[1:2, 64:128], W1[:])
            W2d = cst.tile([128, 128], f32, tag="W2d")
            nc.vector.memset(W2d[:], 0.0)
            nc.sync.dma_start(W2d[0:64, 0:64], W2[:])
            nc.sync.dma_start(W2d[64:128, 64:128], W2[:])
            W3d = cst.tile([128, 128], f32, tag="W3d")
            nc.vector.memset(W3d[:], 0.0)
            nc.sync.dma_start(W3d[0:64, 0:64], W3[:])
            nc.sync.dma_start(W3d[64:128, 64:128], W3[:])
            W4d = cst.tile([128, 32], f32)
            nc.vector.memset(W4d[:], 0.0)
            nc.sync.dma_start(W4d[0:64, 0:1], W4[:])
            nc.sync.dma_start(W4d[64:128, 1:2], W4[:])
            W2db = cst.tile([128, 128], bf16, tag="W2db")
            nc.vector.tensor_copy(W2db[:], W2d[:])
            W3db = cst.tile([128, 128], bf16, tag="W3db")
            nc.vector.tensor_copy(W3db[:], W3d[:])
            W4db = cst.tile([128, 32], bf16, tag="W4db")
            nc.vector.tensor_copy(W4db[:], W4d[:])
            b1d = cst.tile([128, 1], f32, tag="b1d")
            nc.sync.dma_start(b1d[0:64, :], b1[:])
            nc.sync.dma_start(b1d[64:128, :], b1[:])
            b2d = cst.tile([128, 1], f32, tag="b2d")
            nc.sync.dma_start(b2d[0:64, :], b2[:])
            nc.sync.dma_start(b2d[64:128, :], b2[:])
            b3d = cst.tile([128, 1], f32, tag="b3d")
            nc.sync.dma_start(b3d[0:64, :], b3[:])
            nc.sync.dma_start(b3d[64:128, :], b3[:])
            b4b = cst.tile([128, 1], f32, tag="b4b")
            nc.sync.dma_start(b4b[:], b4[:].to_broadcast((128, 1)))
            bgb = cst.tile([128, 1], f32, tag="bgb")
            nc.sync.dma_start(bgb[:], bg[:].to_broadcast((128, 1)))

            ones_row = cst.tile([1, 128], f32, tag="ones_row")
            nc.vector.memset(ones_row[:], 1.0)
            ones_col = cst.tile([128, 1], f32, tag="ones_col")
            nc.vector.memset(ones_col[:], 1.0)

            lensc = cst.tile([128, NSEQ], f32, tag="lensc")
            nc.sync.dma_start(lensc[:], lens_col[:])
            cosMs = cst.tile([128, 8, J + 3], f32, tag="cosMs")
            nc.sync.dma_start(cosMs[:], cosM[:])
            recjs = cst.tile([1, J - 1], f32, tag="recjs")
            nc.sync.dma_start(recjs[:], recj[:])
            facjs = cst.tile([1, J - 1], f32, tag="facjs")
            nc.sync.dma_start(facjs[:], facj[:])

            # t rows / t columns
            trows = []
            for s in range(NSEQ):
                tr = cst.tile([1, L], f32, tag=f"trow{s}")
                nc.sync.dma_start(tr[:], tseq[s : s + 1, :])
                trows.append(tr)
            tcols = cst.tile([128, 2 * NSEQ], f32, tag="tcols")
            for s in range(NSEQ):
                for blk in range(2):
                    nc.sync.dma_start(
                        tcols[:, 2 * s + blk : 2 * s + blk + 1],
                        tseq[s : s + 1, 128 * blk : 128 * blk + 128],
                    )

            # static triangular masks (k < i)
            onesb1 = cst.tile([128, 256], f32, tag="onesb1")
            nc.vector.memset(onesb1[:], 1.0)
            maskb0 = cst.tile([128, 128], f32, tag="maskb0")
            nc.gpsimd.affine_select(maskb0[:], onesb1[:, 0:128], [[-1, 128]],
                                    ALU.is_ge, 0.0, base=-1, channel_multiplier=1)
            maskb1 = cst.tile([128, 256], f32, tag="maskb1")
            nc.gpsimd.affine_select(maskb1[:], onesb1[:], [[-1, 256]],
                                    ALU.is_ge, 0.0, base=127, channel_multiplier=1)

            # iota over event index i (value = p + 128*blk)
            iota_i = cst.tile([128, 2], i32, tag="iota_i")
            nc.gpsimd.iota(iota_i[:], [[128, 2]], base=0, channel_multiplier=1)
            iota_if = cst.tile([128, 2], f32, tag="iota_if")
            nc.vector.tensor_copy(iota_if[:], iota_i[:])
            iota_r = cst.tile([1, 256], i32, tag="iota_r")
            nc.gpsimd.iota(iota_r[:], [[1, 256]], base=0, channel_multiplier=0)
            iota_rf = cst.tile([1, 256], f32, tag="iota_rf")
            nc.vector.tensor_copy(iota_rf[:], iota_r[:])

            # ---------------- pair construction ----------------
            # persistent per-seq 2D pair tiles (i on partitions, k on free)
            X2b0, X2b1 = [], []
            for s in range(NSEQ):
                tbb = pp.tile([128, 256], f32, tag="mm", name=f"tbb{s}")
                nc.tensor.matmul(tbb[:], ones_row[:], trows[s][:],
                                 start=True, stop=True)
                x0 = x2dp.tile([128, 128], f32, tag=f"x2b0_{s}", name=f"x2b0_{s}",
                               bufs=1)
                nc.vector.tensor_scalar(x0[:], tbb[:, 0:128],
                                        tcols[:, 2 * s : 2 * s + 1], -1.0,
                                        ALU.subtract, ALU.mult)
                x1 = x2dp.tile([128, 256], f32, tag=f"x2b1_{s}", name=f"x2b1_{s}",
                               bufs=1)
                nc.vector.tensor_scalar(x1[:], tbb[:, 0:256],
                                        tcols[:, 2 * s + 1 : 2 * s + 2], -1.0,
                                        ALU.subtract, ALU.mult)
                X2b0.append(x0)
                X2b1.append(x1)

            # Z4 2D tiles (per sequence)
            Z4b0 = [cst.tile([128, 128], f32, tag=f"z4b0_{s}", name=f"Z4b0_{s}")
                    for s in range(NSEQ)]
            Z4b1 = [cst.tile([128, 256], f32, tag=f"z4b1_{s}", name=f"Z4b1_{s}")
                    for s in range(NSEQ)]

            # ---------------- MLP groups ----------------
            # group = 2048 pairs loaded just-in-time into an [2, 1024] strip
            # from the persistent X2d tiles; each group yields two L4 outputs
            # [2, 512] (one per 512-dual u).
            # xspec per row j: (src_tile, row_start, n_rows)
            # zspec per (u, j): (dst_tile, row_start, n_rows)
            groups = []  # (xspecs, zspecs)
            for s in range(NSEQ):
                for g in range(8):
                    groups.append((
                        [(X2b0[s], 64 * j + 8 * g, 8) for j in range(2)],
                        [(Z4b0[s], 64 * j + 8 * g + 4 * u, 4)
                         for u in range(2) for j in range(2)]))
                for h in range(2):
                    for g in range(8):
                        groups.append((
                            [(X2b1[s], 64 * h + 32 * j + 4 * g, 4)
                             for j in range(2)],
                            [(Z4b1[s], 64 * h + 32 * j + 4 * g + 2 * u, 2)
                             for u in range(2) for j in range(2)]))

            nodesb = cst.tile([2, 512], f32, tag="nodesb")
            nc.sync.dma_start(nodesb[:], node_xs[:])

            tanh_insts = []

            def mlp_group(rhs_fn, width, zq, slots, precise=False):
                # slots: psum partition bases (one per 512-dual) in zq
                # precise: all-fp32 (node strip feeding the integral term);
                # otherwise L2..L4 run with bf16 operands (fp32 accumulate).
                nu = width // 512
                hdt = f32 if precise else bf16
                w2, w3, w4 = ((W2d, W3d, W4d) if precise
                              else (W2db, W3db, W4db))
                pm1 = pp.tile([128, 1024], f32, tag="mm")
                for u in range(nu):
                    nc.tensor.matmul(pm1[:, 512 * u : 512 * u + 512], W1d[:],
                                     rhs_fn(u), start=True, stop=True)
                H1 = hp.tile([128, 1024], hdt, tag="h1")
                tanh_insts.append(nc.scalar.activation(
                    H1[:, 0:width], pm1[:, 0:width], AF.Tanh, bias=b1d[:]))
                pm2 = pp.tile([128, 1024], f32, tag="mm")
                for u in range(nu):
                    nc.tensor.matmul(pm2[:, 512 * u : 512 * u + 512], w2[:],
                                     H1[:, 512 * u : 512 * u + 512],
                                     start=True, stop=True)
                H2 = hp.tile([128, 1024], hdt, tag="h2")
                tanh_insts.append(nc.scalar.activation(
                    H2[:, 0:width], pm2[:, 0:width], AF.Tanh, bias=b2d[:]))
                pm3 = pp.tile([128, 1024], f32, tag="mm")
                for u in range(nu):
                    nc.tensor.matmul(pm3[:, 512 * u : 512 * u + 512], w3[:],
                                     H2[:, 512 * u : 512 * u + 512],
                                     start=True, stop=True)
                H3 = hp.tile([128, 1024], hdt, tag="h3")
                tanh_insts.append(nc.scalar.activation(
                    H3[:, 0:width], pm3[:, 0:width], AF.Tanh, bias=b3d[:]))
                l4s = []
                for u in range(nu):
                    p0 = slots[u]
                    l4s.append(nc.tensor.matmul(
                        zq[p0 : p0 + 32, 512 * u : 512 * u + 512], w4[:],
                        H3[:, 512 * u : 512 * u + 512],
                        start=True, stop=True))
                return l4s

            # triples: 3 groups share one [66, 1024] psum z tile
            # (partition slots 0/32/64, one col half per 512-dual)
            assert len(groups) % 3 == 0
            zs_copies = []
            last_l4 = None
            for tri in range(len(groups) // 3):
                zq = pzp.tile([96, 1024], f32, tag="z", name=f"zq{tri}")
                gtri = groups[3 * tri : 3 * tri + 3]
                for gi, (xspecs, zspecs) in enumerate(gtri):
                    xg = x2dp.tile([2, 1024], f32, tag="xg", bufs=3,
                                   name=f"xg{tri}_{gi}")
                    for j, (srct, r0, nr) in enumerate(xspecs):
                        nc.sync.dma_start(xg[j : j + 1, :], srct[r0 : r0 + nr, :])
                    l4s = mlp_group(
                        lambda u, xg=xg:
                            xg[:, 512 * u : 512 * u + 512],
                        1024, zq, slots=(32 * gi, 32 * gi))
                    last_l4 = l4s[-1]
                zs = fbp.tile([96, 1024], f32, tag="zs", name=f"zs{tri}")
                zs_copies.append(nc.vector.tensor_copy(zs[:], zq[:]))
                for gi, (xspecs, zspecs) in enumerate(gtri):
                    for k, (dst, r0, nr) in enumerate(zspecs):
                        u, j = divmod(k, 2)
                        src_row = 32 * gi + j
                        cs = slice(512 * u, 512 * u + 512)
                        nc.sync.dma_start(dst[r0 : r0 + nr, :],
                                          zs[src_row : src_row + 1, cs])

            # node strip through the same MLP (stream A only is used)
            zqn = pzp.tile([96, 1024], f32, tag="z", name="zqn")
            l4s = mlp_group(lambda u: nodesb[:], 512, zqn, slots=(0,), precise=True)
            last_l4 = l4s[-1]
            zsn = fbp.tile([96, 1024], f32, tag="zs", name="zsn")
            zs_copies.append(nc.vector.tensor_copy(zsn[0:2, 0:512], zqn[0:2, 0:512]))
            # chain zs copies in DVE stream, then gate all epilogue DVE/PE work
            for a, b in zip(zs_copies, zs_copies[1:]):
                add_dep_helper(b.ins, a.ins, sync=False, reason="zs chain")
            dve_gate = zs_copies[-1]
            pe_gate = last_l4

            _orig_mm = nc.tensor.matmul
            def g_mm(*a, **kw):
                inst = _orig_mm(*a, **kw)
                add_dep_helper(inst.ins, pe_gate.ins, sync=False,
                               reason="PE: epilogue after MLP")
                return inst

            def g_dve(op, *a, **kw):
                inst = op(*a, **kw)
                add_dep_helper(inst.ins, dve_gate.ins, sync=False,
                               reason="DVE: epilogue after MLP")
                return inst

            # ---------------- softplus phase ----------------
            # softplus(z+b4) = max(z+b4, 0) + Ln(1 + Exp(-|z+b4|))
            # (no Softplus table on this toolchain; Exp shares the tanh
            #  table and Ln/Exp share natural_log_exp_and_others)
            softplus_insts = []

            sp_ctr = [0]

            def softplus_block(Z, P, F, b4ap):
                sp_ctr[0] += 1
                n = sp_ctr[0]
                t1 = fbp.tile([128, 512], f32, tag="sp_t1", name=f"spt1_{n}")
                softplus_insts.append(nc.scalar.activation(
                    t1[0:P, 0:F], Z[0:P, 0:F], AF.Abs, bias=b4ap))
                e = fbp.tile([128, 512], f32, tag="sp_e", name=f"spe_{n}")
                softplus_insts.append(nc.scalar.activation(
                    e[0:P, 0:F], t1[0:P, 0:F], AF.Exp, scale=-1.0))
                lg = fbp.tile([128, 512], f32, tag="sp_l", name=f"spl_{n}")
                softplus_insts.append(nc.scalar.activation(
                    lg[0:P, 0:F], e[0:P, 0:F], AF.Ln, bias=1.0))
                zm = fbp.tile([128, 512], f32, tag="sp_zm", name=f"spzm_{n}")
                g_dve(nc.vector.tensor_scalar, zm[0:P, 0:F], Z[0:P, 0:F],
                      b4ap, 0.0, ALU.add, ALU.max)
                fb = fbp.tile([128, 512], f32, tag="sp_fb", name=f"spfb_{n}")
                g_dve(nc.vector.tensor_tensor, fb[0:P, 0:F], zm[0:P, 0:F],
                      lg[0:P, 0:F], ALU.add)
                return fb

            fvt = softplus_block(zsn, 2, 512, b4b[0:2, :])
            fv = cst.tile([2, 512], f32, tag="fv")
            g_dve(nc.vector.tensor_copy, fv[:], fvt[0:2, 0:512])

            lamb = cst.tile([128, 2 * NSEQ], f32, tag="lamb")
            for s in range(NSEQ):
                for blk in range(2):
                    Z = Z4b0[s] if blk == 0 else Z4b1[s]
                    m = maskb0 if blk == 0 else maskb1
                    F = 128 if blk == 0 else 256
                    fb = softplus_block(Z, 128, F, b4b[:])
                    sc = fbp.tile([128, 256], f32, tag="fbs")
                    g_dve(nc.vector.scalar_tensor_tensor,
                        sc[:, 0:F], fb[:, 0:F], 1.0, m[:], ALU.mult, ALU.mult,
                        accum_out=lamb[:, 2 * s + blk : 2 * s + blk + 1])

            for sp in softplus_insts:
                add_dep_helper(sp.ins, tanh_insts[-1].ins, sync=False,
                               reason="ACT table phase: tanh before softplus")

            # ---------------- log-lambda ----------------
            vm = cst.tile([128, 2 * NSEQ], f32, tag="vm")
            for s in range(NSEQ):
                for blk in range(2):
                    c = 2 * s + blk
                    g_dve(nc.vector.tensor_scalar, vm[:, c : c + 1],
                                            iota_if[:, blk : blk + 1],
                                            lensc[:, s : s + 1], None, ALU.is_lt)
            vmc = cst.tile([128, 2 * NSEQ], f32, tag="vmc")
            g_dve(nc.vector.tensor_scalar, vmc[:], vm[:], -1.0, 1.0, ALU.mult, ALU.add)
            lamb2 = cst.tile([128, 2 * NSEQ], f32, tag="lamb2")
            g_dve(nc.vector.tensor_scalar, lamb2[:], lamb[:], bgb[:], None, ALU.add)
            lambm = cst.tile([128, 2 * NSEQ], f32, tag="lambm")
            g_dve(nc.vector.scalar_tensor_tensor, lambm[:], lamb2[:], 1.0, vm[:],
                                           ALU.mult, ALU.mult)
            g_dve(nc.vector.tensor_tensor, lambm[:], lambm[:], vmc[:], ALU.add)
            lnl = cst.tile([128, 2 * NSEQ], f32, tag="lnl")
            lnacc = cst.tile([128, 1], f32, tag="lnacc")
            ln_inst = nc.scalar.activation(lnl[:], lambm[:], AF.Ln,
                                           accum_out=lnacc[:])
            add_dep_helper(ln_inst.ins, softplus_insts[-1].ins, sync=False,
                           reason="ACT table phase: softplus before ln")
            psl = pzp.tile([96, 1024], f32, tag="z", name="psl")
            g_mm(psl[0:1, 0:1], lnacc[:], ones_col[:],
                             start=True, stop=True)
            slcell = cst.tile([1, 1], f32, tag="slcell")
            g_dve(nc.vector.tensor_copy, slcell[:], psl[0:1, 0:1])

            # ---------------- chebyshev coefficients ----------------
            fv_col = cst.tile([128, 8], f32, tag="fv_col")
            nc.sync.dma_start(fv_col[:], fv[:])
            pc = pzp.tile([96, 1024], f32, tag="z", name="pc")
            for q in range(8):
                g_mm(pc[0:1, 0 : J + 3], fv_col[:, q : q + 1],
                                 cosMs[:, q, :], start=(q == 0),
                                 stop=(q == 7))
            csb = cst.tile([1, J + 3], f32, tag="csb")
            g_dve(nc.vector.tensor_copy, csb[:], pc[0:1, 0 : J + 3])
            cdiff = cst.tile([1, J - 1], f32, tag="cdiff")
            g_dve(nc.vector.tensor_tensor, cdiff[:], csb[:, 0 : J - 1], csb[:, 2 : J + 1],
                                    ALU.subtract)
            cint_row = cst.tile([1, J], f32, tag="cint_row")
            g_dve(nc.vector.scalar_tensor_tensor, cint_row[:, 1:J], cdiff[:], CON,
                                           recjs[:], ALU.mult, ALU.mult)
            c0t = cst.tile([1, J - 1], f32, tag="c0t")
            g_dve(nc.vector.tensor_tensor, c0t[:], cint_row[:, 1:J], facjs[:], ALU.mult)
            # slot 0 of cint = 0.5*c0 = sum(fac*tail); T_0 row of ladder is ones
            g_dve(nc.vector.tensor_reduce, cint_row[:, 0:1], c0t[:],
                                    mybir.AxisListType.X, ALU.add)
            cint_col = cst.tile([128, 1], f32, tag="cint_col")
            nc.sync.dma_start(cint_col[:], cint_row[:])

            # ---------------- events -> y ----------------
            t_row = cst.tile([1, 1024], f32, tag="t_row")
            nc.sync.dma_start(t_row[:], tseq[:])
            vm_row = cst.tile([1, 1024], f32, tag="vm_row")
            for s in range(NSEQ):
                g_dve(nc.vector.tensor_scalar, vm_row[:, 256 * s : 256 * s + 256],
                                        iota_rf[:], lensc[0:1, s : s + 1], None,
                                        ALU.is_lt)
            ty = cst.tile([1, 1024], f32, tag="ty")
            g_dve(nc.vector.tensor_scalar, ty[:], t_row[:], -2.0 / T_END, 2.0,
                                    ALU.mult, ALU.add)
            ty2 = cst.tile([1, 1024], f32, tag="ty2")
            g_dve(nc.vector.scalar_tensor_tensor, ty2[:], ty[:], 1.0, vm_row[:],
                                           ALU.mult, ALU.mult)
            y_row = cst.tile([1, 1024], f32, tag="y_row")
            g_dve(nc.vector.tensor_scalar, y_row[:], ty2[:], 1.0, None, ALU.subtract)

            # ---------------- T ladder ----------------
            TL = cst.tile([128, 1024], f32, tag="TL")
            g_dve(nc.vector.memset, TL[0:1, :], 1.0)
            nc.gpsimd.dma_start(TL[1:2, :], y_row[:])
            S64 = cst.tile([64, 63], f32, tag="S64")
            ones64 = cst.tile([64, 64], f32, tag="ones64")
            g_dve(nc.vector.memset, ones64[:], 1.0)
            nc.gpsimd.affine_select(S64[:], ones64[:, 0:63], [[-1, 63]],
                                    ALU.is_equal, 0.0, base=-1,
                                    channel_multiplier=1)
            prev_row = y_row
            for m in (2, 4, 8, 16, 32, 64):
                tsq = ladp.tile([1, 1024], f32, tag="tsq")
                g_dve(nc.vector.scalar_tensor_tensor, tsq[:], prev_row[:], 2.0,
                                               prev_row[:], ALU.mult, ALU.mult)
                tm_row = ladp.tile([1, 1024], f32, tag="tmr")
                g_dve(nc.vector.tensor_scalar, tm_row[:], tsq[:], 1.0, None,
                                        ALU.subtract)
                nc.gpsimd.dma_start(TL[m : m + 1, :], tm_row[:])
                n_new = m - 1
                if n_new >= 1:
                    anti = ladp.tile([64, 63], f32, tag="anti")
                    nc.gpsimd.affine_select(anti[0:m, 0:n_new],
                                            ones64[0:m, 0:n_new], [[1, n_new]],
                                            ALU.is_equal, 0.0, base=-(m - 1),
                                            channel_multiplier=1)
                    B = pp.tile([64, 1024], f32, tag="mm")
                    Sh = pp.tile([64, 1024], f32, tag="mm")
                    Rv = pp.tile([64, 1024], f32, tag="mm")
                    for u in range(2):
                        cs = slice(512 * u, 512 * u + 512)
                        g_mm(B[0:n_new, cs],
                                         ones_row[0:1, 0:n_new],
                                         tm_row[:, cs], start=True, stop=True)
                        g_mm(Sh[0:n_new, cs],
                                         S64[0:m, 0:n_new],
                                         TL[0:m, cs], start=True, stop=True)
                        g_mm(Rv[0:n_new, cs],
                                         anti[0:m, 0:n_new],
                                         TL[0:m, cs], start=True, stop=True)
                    Bs = ladp.tile([64, 1024], f32, tag="Bs")
                    g_dve(nc.vector.tensor_copy, Bs[0:n_new, :], B[0:n_new, :])
                    newr = ladp.tile([64, 1024], f32, tag="newr")
                    g_dve(nc.vector.scalar_tensor_tensor, newr[0:n_new, :],
                                                   Sh[0:n_new, :], 2.0,
                                                   Bs[0:n_new, :], ALU.mult,
                                                   ALU.mult)
                    g_dve(nc.vector.tensor_tensor, newr[0:n_new, :], newr[0:n_new, :],
                                            Rv[0:n_new, :], ALU.subtract)
                    nc.gpsimd.dma_start(TL[m + 1 : 2 * m, :], newr[0:n_new, :])
                prev_row = tm_row

            # ---------------- integral dot + final ----------------
            pdot = pp.tile([128, 1024], f32, tag="mm", name="pdot")
            for u in range(2):
                cs = slice(512 * u, 512 * u + 512)
                g_mm(pdot[0:1, cs], cint_col[:], TL[:, cs],
                                 start=True, stop=True)
            sdot = cst.tile([1, 1], f32, tag="sdot")
            g_dve(nc.vector.tensor_reduce, sdot[:], pdot[0:1, :], mybir.AxisListType.X,
                                    ALU.add)
            int_cell = cst.tile([1, 1], f32, tag="int_cell")
            g_dve(nc.vector.scalar_tensor_tensor, int_cell[:], bgb[0:1, :],
                                           float(NSEQ) * T_END, sdot[:],
                                           ALU.mult, ALU.add)
            res = cst.tile([1, 1], f32, tag="res")
            g_dve(nc.vector.tensor_tensor, res[:], slcell[:], int_cell[:], ALU.subtract)
            nc.sync.dma_start(out_part[:], res[:])

    nc.compile()
    return nc


def prep_inputs(inputs):
    """Host-side sharding/layout prep. Returns per-core input maps."""
    seq_pads = np.asarray(inputs["seq_pads"], np.float32)
    seq_lens = np.asarray(inputs["seq_lens"])
    t = seq_pads[:, :, 0]  # [32, 256]

    theta = (np.pi * (np.arange(N_NODES) + 0.5) / N_NODES).astype(np.float64)
    xk = (0.5 * T_END * (np.cos(theta) + 1.0))
    node_xs = np.zeros(1024, np.float32)
    node_xs[:N_NODES] = xk.astype(np.float32)
    node_xs = node_xs.reshape(2, 512)

    # cosM[p, q, j] = (2/N) * cos(j * theta_n), n = 8p + q  (0 for n >= N)
    n_idx = (8 * np.arange(128)[:, None] + np.arange(8)[None, :])  # [128, 8]
    jj = np.arange(J + 3)
    theta_n = np.pi * (np.minimum(n_idx, N_NODES - 1) + 0.5) / N_NODES  # [128, 8]
    cosM = (2.0 / N_NODES) * np.cos(jj[None, None, :] * theta_n[..., None])
    cosM[n_idx >= N_NODES, :] = 0.0
    cosM = cosM.astype(np.float32)

    recj = (1.0 / np.arange(1, J)).astype(np.float32).reshape(1, J - 1)
    facj = np.where(np.arange(J - 1) % 2 == 0, 1.0, -1.0).astype(np.float32).reshape(1, J - 1)

    common = {
        "W1": np.asarray(inputs["W1"], np.float32),
        "b1": np.asarray(inputs["b1"], np.float32),
        "W2": np.asarray(inputs["W2"], np.float32),
        "b2": np.asarray(inputs["b2"], np.float32),
        "W3": np.asarray(inputs["W3"], np.float32),
        "b3": np.asarray(inputs["b3"], np.float32),
        "W4": np.asarray(inputs["W4"], np.float32),
        "b4": np.asarray(inputs["b4"], np.float32),
        "background": np.asarray(inputs["background"], np.float32),
        "node_xs": node_xs, "cosM": cosM, "recj": recj, "facj": facj,
    }
    in_maps = []
    for c in range(8):
        sl = slice(4 * c, 4 * c + 4)
        m = dict(common)
        m["tseq"] = np.ascontiguousarray(t[sl])
        m["lens_col"] = np.broadcast_to(
            seq_lens[sl].astype(np.float32)[None, :], (128, 4)).copy()
        in_maps.append(m)
    return in_maps


def kernel(**inputs) -> np.ndarray:
    from concourse.bass_utils import run_bass_kernel_spmd

    nc = build_program()
    in_maps = prep_inputs(inputs)
    res = run_bass_kernel_spmd(nc, in_maps, core_ids=list(range(8)))
    partials = [r["out_part"][0, 0] for r in res.results]
    total = np.float32(0.0)
    for p in partials:
        total = np.float32(total + np.float32(p))
    return np.asarray(-total / np.float32(32.0), dtype=np.float32)

